# revision 4
# baseline (speedup 1.0000x reference)
"""Trainium2 Bass kernel for nn_RelPosRFFBias — factorized Fourier, v5 (full host mirror).

Math: per head h, bias(t,s) = g_h(|c_t - c_s|) with g_h fit as a ~125-tone
cosine+sine series.  In sorted-center order, for t >= s (lower triangle):

  g(d) = sum_k a_k cos(w_k d) + b_k sin(w_k d)
       = sum_k cos_t (a_k cos_s - b_k sin_s) + sin_t (a_k sin_s + b_k cos_s)

so ONE rank-256 matmul per (row-tile, head) with lhs = raw interleaved
cos/sin table U and rhs = rcomb = aq (.) U + bq (.) V (V = pair-swapped U)
evaluates the whole lower block-triangle, diagonal tiles included.  The host
mirrors the strict upper triangle (inter- and intra-tile) by symmetry and
undoes the sort permutation.

v5 vs v4 baseline:
 - no separate P matmuls / diag sign fixups: PE work halves (43us -> ~18us).
 - builds are 2 DVE passes per (head, chunk): tensor_scalar + fused
   scalar_tensor_tensor, both 4x-mode eligible.
 - PSUM evacuation split across Scalar (i=3, i=1) and GpSimd (i=2, i=0).
 - warmup trimmed to ~7 matmuls (p-state ramp needs ~3us, not 6.2us).
 - head-group-column-major schedule: output DMA per (row-tile, 4-head group)
   spread across SP/Act/DVE queues; short tail.
"""

import math

import numpy as np

B, T = 8, 512
RFF, NH = 16, 16
F_MIN, F_MAX = 2.0, 64.0
TWO_PI = 2.0 * math.pi

N_CORES = 8
L_PER = 1.0625
KU = 124
NQMAX = 128
TILE = 128
NT = T // TILE
FIT_LAM = 1e-5
FIT_ITERS = 14
NWARM = 7

_MODULE = None
_LAST_RESULTS = None
_FIT_CACHE = {}


# ---------------------------------------------------------------- host: fit
def _gelu64(x):
    try:
        from scipy.special import erf
    except ImportError:
        erf = np.vectorize(math.erf)
    return 0.5 * x * (1.0 + erf(x / math.sqrt(2.0)))


def _g_of_D(D, phase, W1, b1, W2, b2, freqs):
    arg = TWO_PI * D[:, None] * freqs[None, :] + phase[None, :]
    feats = np.concatenate([np.sin(arg), np.cos(arg)], axis=-1)
    return _gelu64(feats @ W1 + b1) @ W2 + b2


def _tone_grid():
    freqs = np.logspace(math.log10(F_MIN), math.log10(F_MAX), RFF).astype(np.float64)
    uni = np.arange(KU) / L_PER
    cut = uni[-1]
    cand = sorted(set(
        round(f, 6)
        for f in np.concatenate([(freqs[:, None] + freqs[None, :]).ravel(), 2 * freqs])
        if cut + 0.2 < f < 145.0
    ))
    omQ = np.concatenate([uni, np.asarray(cand[: NQMAX - KU], dtype=np.float64)])
    omQ.sort()
    omP = omQ[1:min(len(omQ), 129)]
    return freqs, omQ, omP


def _fit_coefs(phase, W1, b1, W2, b2):
    freqs, omQ, omP = _tone_grid()
    NG = 32768
    Dg = (np.arange(NG) + 0.5) / NG
    G = _g_of_D(Dg, phase, W1, b1, W2, b2, freqs)
    Phi = np.concatenate(
        [np.cos(Dg[:, None] * TWO_PI * omQ[None, :]),
         np.sin(Dg[:, None] * TWO_PI * omP[None, :])], axis=1)
    lam = FIT_LAM * NG
    w = np.ones(NG)
    best = None
    for _ in range(FIT_ITERS):
        Pw = Phi * w[:, None]
        A = Pw.T @ Phi
        A[np.diag_indices_from(A)] += lam
        coef = np.linalg.solve(A, Pw.T @ G)
        res = np.abs(Phi @ coef - G).max(axis=1)
        mx = res.max()
        if best is None or mx < best[0]:
            best = (mx, coef.copy())
        w = w * (0.05 + res / mx)
        w = np.maximum(w / w.mean(), 1e-6)
    mx, coef = best
    return omQ, omP, coef[: len(omQ)], coef[len(omQ):], mx


# ---------------------------------------------------------------- device
def _build_module():
    import concourse.tile as tile
    from concourse import bacc, mybir
    from contextlib import ExitStack

    f32 = mybir.dt.float32
    bf16 = mybir.dt.bfloat16
    Alu = mybir.AluOpType
    Act = mybir.ActivationFunctionType

    nc = bacc.Bacc("TRN2", target_bir_lowering=False, debug=False)

    # packU = [U | V] bf16: U[p, c*512+s], V = pair-swapped U
    packU_d = nc.dram_tensor("packU", [TILE, 4 * T], bf16, kind="ExternalInput")
    # packC f32: cols 0:32 = aq (col 2h+c), cols 32:64 = bq
    packC_d = nc.dram_tensor("packC", [TILE, 4 * NH], f32, kind="ExternalInput")
    # compact block-lower-triangle outputs, head-major per row-tile i
    out_ds = [nc.dram_tensor(f"out{i}", [TILE, NH * (i + 1) * TILE], bf16,
                             kind="ExternalOutput") for i in range(NT)]

    with tile.TileContext(nc) as tc:
        with ExitStack() as ctx:
            const = ctx.enter_context(tc.tile_pool(name="const", bufs=1))
            rhspool = ctx.enter_context(tc.tile_pool(name="rhs", bufs=1))
            tmppool = ctx.enter_context(tc.tile_pool(name="tmp", bufs=6))
            stpool = ctx.enter_context(tc.tile_pool(name="stage", bufs=1))
            # PSUM: pb = [128,1024] (2 banks) x2 bufs for i3/i2 head pairs;
            # pv = [128,1536] (3 banks) x1 buf for i1 pairs + i0;
            # pw = [128,512] x1 buf for warmup.  4+3+1 = 8 banks.
            pb = ctx.enter_context(tc.tile_pool(name="pb", bufs=2, space="PSUM"))
            pv = ctx.enter_context(tc.tile_pool(name="pv", bufs=1, space="PSUM"))
            pw = ctx.enter_context(tc.tile_pool(name="pw", bufs=1, space="PSUM"))

            packU = const.tile([TILE, 4 * T], bf16, tag="packU")
            nc.sync.dma_start(packU[:], packU_d.ap())
            packC = const.tile([TILE, 4 * NH], f32, tag="packC")
            nc.scalar.dma_start(packC[:], packC_d.ap())

            U = [packU[:, c * T:(c + 1) * T] for c in range(2)]
            V = [packU[:, (2 + c) * T:(3 + c) * T] for c in range(2)]
            aq = packC[:, 0:2 * NH]
            bq = packC[:, 2 * NH:4 * NH]

            # PE warm-up: ~3us of dummy matmuls to finish the p-state ramp
            # while the input DMA + first builds land.
            warm_sb = const.tile([TILE, T], bf16, tag="warm")
            nc.vector.memset(warm_sb[:], 0)
            for _ in range(NWARM):
                warm_ps = pw.tile([TILE, T], f32, tag="pw")
                nc.tensor.matmul(warm_ps[:], warm_sb[:, 0:TILE], warm_sb[:],
                                 start=True, stop=True)

            # rcomb: [128, NH*1024], per head h chunk c at h*1024 + c*512
            rcomb = rhspool.tile([TILE, NH * 2 * T], bf16, tag="rcomb",
                                 name="rcomb")

            def build(h, pool_c0=False):
                for c in range(2):
                    col = 2 * h + c
                    tmp = tmppool.tile([TILE, T], bf16, tag="tmp")
                    eng = nc.gpsimd if (pool_c0 and c == 0) else nc.vector
                    eng.tensor_scalar(
                        tmp[:], V[c], bq[:, col:col + 1], None, Alu.mult)
                    nc.vector.scalar_tensor_tensor(
                        rcomb[:, h * 2 * T + c * T: h * 2 * T + (c + 1) * T],
                        U[c], aq[:, col:col + 1], tmp[:], Alu.mult, Alu.add)

            for h in range(4):
                build(h, pool_c0=(h >= 2))

            stages = [stpool.tile([TILE, NH * (i + 1) * TILE], bf16,
                                  tag=f"stage{i}", name=f"stage{i}")
                      for i in range(NT)]
            rc_v = rcomb[:].rearrange("p (h x) -> p h x", h=NH)

            def rslice(h, c, n):
                return rcomb[:, h * 2 * T + c * T: h * 2 * T + c * T + n]

            W2_ = 3 * TILE
            W1_ = 2 * TILE
            for g in range(4):
                h0 = 4 * g
                # i = 3: head pairs, 2 banks per psum tile, N=512 each
                for hp in (h0, h0 + 2):
                    ps = pb.tile([TILE, 2 * T], f32, tag="pb")
                    for j in range(2):
                        for c in range(2):
                            nc.tensor.matmul(
                                ps[:, j * T:(j + 1) * T],
                                packU[:, c * T + 3 * TILE: c * T + 4 * TILE],
                                rslice(hp + j, c, T),
                                start=(c == 0), stop=(c == 1),
                                skip_group_check=True)
                    nc.scalar.activation(
                        stages[3][:, hp * T:(hp + 2) * T], ps[:], Act.Identity)
                # builds for the next head group while PE chews on i1/i0/i2
                if g < 3:
                    for h in range(h0 + 4, h0 + 8):
                        build(h, pool_c0=True)
                # i = 1 (two heads per bank) + i = 0 (four heads in one bank)
                psv = pv.tile([TILE, 3 * T], f32, tag="pv")
                for k, hp in enumerate((h0, h0 + 2)):
                    for c in range(2):
                        nc.tensor.matmul(
                            psv[:, k * T:(k + 1) * T],
                            packU[:, c * T + TILE: c * T + 2 * TILE],
                            rc_v[:, hp:hp + 2, c * T: c * T + W1_],
                            start=(c == 0), stop=(c == 1),
                            skip_group_check=True)
                for c in range(2):
                    nc.tensor.matmul(
                        psv[:, 2 * T:3 * T],
                        packU[:, c * T: c * T + TILE],
                        rc_v[:, h0:h0 + 4, c * T: c * T + TILE],
                        start=(c == 0), stop=(c == 1),
                        skip_group_check=True)
                nc.vector.tensor_copy(
                    stages[1][:, h0 * W1_:(h0 + 4) * W1_], psv[:, 0:2 * T])
                nc.vector.tensor_copy(
                    stages[0][:, h0 * TILE:(h0 + 4) * TILE], psv[:, 2 * T:3 * T])
                nc.sync.dma_start(
                    out_ds[3].ap()[:, h0 * T:(h0 + 4) * T],
                    stages[3][:, h0 * T:(h0 + 4) * T])
                # i = 2: head pairs, N=384 into each bank of a 2-bank tile
                for hp in (h0, h0 + 2):
                    ps = pb.tile([TILE, 2 * T], f32, tag="pb")
                    for j in range(2):
                        for c in range(2):
                            nc.tensor.matmul(
                                ps[:, j * T:j * T + W2_],
                                packU[:, c * T + 2 * TILE: c * T + 3 * TILE],
                                rslice(hp + j, c, W2_),
                                start=(c == 0), stop=(c == 1),
                                skip_group_check=True)
                    nc.scalar.activation(
                        stages[2][:, hp * W2_:(hp + 2) * W2_]
                        .rearrange("p (b s) -> p b s", b=2),
                        ps[:].rearrange("p (b s) -> p b s", b=2)[:, :, 0:W2_],
                        Act.Identity)
                nc.scalar.dma_start(
                    out_ds[1].ap()[:, h0 * W1_:(h0 + 4) * W1_],
                    stages[1][:, h0 * W1_:(h0 + 4) * W1_])
                nc.sync.dma_start(
                    out_ds[0].ap()[:, h0 * TILE:(h0 + 4) * TILE],
                    stages[0][:, h0 * TILE:(h0 + 4) * TILE])
                nc.sync.dma_start(
                    out_ds[2].ap()[:, h0 * W2_:(h0 + 4) * W2_],
                    stages[2][:, h0 * W2_:(h0 + 4) * W2_])

    nc.compile()
    return nc


# ---------------------------------------------------------------- host glue
def _to_bf16(x):
    import ml_dtypes
    return np.ascontiguousarray(x, np.float32).astype(ml_dtypes.bfloat16)


def _host_tables(c_sorted, omQ):
    """U: [128, 1024] interleaved cos/sin, chunk-major; V: pair-swapped."""
    nQ = len(omQ)
    ang = np.multiply.outer(omQ, c_sorted.astype(np.float64)) * TWO_PI  # [nQ, T]
    cosr = np.cos(ang).astype(np.float32)
    sinr = np.sin(ang).astype(np.float32)
    U = np.zeros((TILE, 2 * T), np.float32)
    for c in range(2):
        for kk in range(64):
            k = 64 * c + kk
            if k >= nQ:
                break
            U[2 * kk, c * T:(c + 1) * T] = cosr[k]
            U[2 * kk + 1, c * T:(c + 1) * T] = sinr[k]
    Vt = np.zeros_like(U)
    Vt[0::2] = U[1::2]
    Vt[1::2] = U[0::2]
    return U, Vt


def _coef_cols(a, b, nQ):
    """packC [128, 64] f32: cols 0:32 aq (col 2h+c), 32:64 bq (-b even, +b odd)."""
    a_pad = np.zeros((TILE, NH), np.float64)
    a_pad[:nQ] = a
    b_pad = np.zeros((TILE, NH), np.float64)
    b_pad[1:1 + b.shape[0]] = b
    pc = np.zeros((TILE, 4 * NH), np.float32)
    for h in range(NH):
        for c in range(2):
            col = 2 * h + c
            for kk in range(64):
                k = 64 * c + kk
                pc[2 * kk, col] = a_pad[k, h]
                pc[2 * kk + 1, col] = a_pad[k, h]
                pc[2 * kk, 2 * NH + col] = -b_pad[k, h]
                pc[2 * kk + 1, 2 * NH + col] = b_pad[k, h]
    return pc


def kernel(centers01, mask, bias_phase, W1, b1, W2, b2):
    global _MODULE, _LAST_RESULTS
    from concourse.bass_utils import run_bass_kernel_spmd

    centers01 = np.asarray(centers01, np.float32)
    bias_phase = np.asarray(bias_phase, np.float64)
    W1 = np.asarray(W1, np.float64)
    b1 = np.asarray(b1, np.float64)
    W2 = np.asarray(W2, np.float64)
    b2 = np.asarray(b2, np.float64)

    ck = hash((bias_phase.tobytes(), W1.tobytes(), b1.tobytes(),
               W2.tobytes(), b2.tobytes()))
    if ck not in _FIT_CACHE:
        _FIT_CACHE[ck] = _fit_coefs(bias_phase, W1, b1, W2, b2)
    omQ, omP, a, b, _gridmax = _FIT_CACHE[ck]
    nQ = len(omQ)

    packC = _coef_cols(a, b, nQ)

    if _MODULE is None:
        _MODULE = _build_module()
    nc = _MODULE

    in_maps = []
    idxs = []
    for bi in range(N_CORES):
        c = centers01[bi]
        idx = np.argsort(c, kind="stable")
        idxs.append(idx)
        U, Vt = _host_tables(c[idx], omQ)
        in_maps.append({
            "packU": _to_bf16(np.concatenate([U, Vt], axis=1)),
            "packC": packC,
        })

    res = run_bass_kernel_spmd(nc, in_maps, list(range(N_CORES)))
    _LAST_RESULTS = res

    out = np.empty((B, NH, T, T), np.float32)
    M = np.empty((NH, T, T), np.float32)
    iu = np.triu_indices(T, 1)
    for bi in range(N_CORES):
        for i in range(NT):
            Wi = (i + 1) * TILE
            raw = np.asarray(res.results[bi][f"out{i}"])
            if raw.dtype != np.uint16:
                raw = raw.view(np.uint16)
            f = (raw.astype(np.uint32) << 16).view(np.float32)
            M[:, i * TILE:(i + 1) * TILE, 0:Wi] = \
                f.reshape(TILE, NH, Wi).transpose(1, 0, 2)
        M[:, iu[0], iu[1]] = M[:, iu[1], iu[0]]
        inv = np.empty(T, np.int64)
        inv[idxs[bi]] = np.arange(T)
        out[bi] = M[:, inv][:, :, inv]
    m = np.asarray(mask, bool)
    if not m.all():
        out *= (m[:, None, :, None] & m[:, None, None, :]).astype(np.float32)
    return out


# revision 9
# speedup vs baseline: 3.0199x; 3.0199x over previous
"""Trainium2 Bass kernel for nn_RelPosRFFBias — factorized Fourier, v5 (full host mirror).

Math: per head h, bias(t,s) = g_h(|c_t - c_s|) with g_h fit as a ~125-tone
cosine+sine series.  In sorted-center order, for t >= s (lower triangle):

  g(d) = sum_k a_k cos(w_k d) + b_k sin(w_k d)
       = sum_k cos_t (a_k cos_s - b_k sin_s) + sin_t (a_k sin_s + b_k cos_s)

so ONE rank-256 matmul per (row-tile, head) with lhs = raw interleaved
cos/sin table U and rhs = rcomb = aq (.) U + bq (.) V (V = pair-swapped U)
evaluates the whole lower block-triangle, diagonal tiles included.  The host
mirrors the strict upper triangle (inter- and intra-tile) by symmetry and
undoes the sort permutation.

v5 vs v4 baseline:
 - no separate P matmuls / diag sign fixups: PE work halves (43us -> ~18us).
 - builds are 2 DVE passes per (head, chunk): tensor_scalar + fused
   scalar_tensor_tensor, both 4x-mode eligible.
 - PSUM evacuation split across Scalar (i=3, i=1) and GpSimd (i=2, i=0).
 - warmup trimmed to ~7 matmuls (p-state ramp needs ~3us, not 6.2us).
 - head-group-column-major schedule: output DMA per (row-tile, 4-head group)
   spread across SP/Act/DVE queues; short tail.
"""

import math

import numpy as np

B, T = 8, 512
RFF, NH = 16, 16
F_MIN, F_MAX = 2.0, 64.0
TWO_PI = 2.0 * math.pi

N_CORES = 8
L_PER = 1.0625
KU = 124
NQMAX = 128
TILE = 128
NT = T // TILE
FIT_LAM = 1e-5
FIT_ITERS = 14
NWARM = 7

_MODULE = None
_LAST_RESULTS = None
_FIT_CACHE = {}


# ---------------------------------------------------------------- host: fit
def _gelu64(x):
    try:
        from scipy.special import erf
    except ImportError:
        erf = np.vectorize(math.erf)
    return 0.5 * x * (1.0 + erf(x / math.sqrt(2.0)))


def _g_of_D(D, phase, W1, b1, W2, b2, freqs):
    arg = TWO_PI * D[:, None] * freqs[None, :] + phase[None, :]
    feats = np.concatenate([np.sin(arg), np.cos(arg)], axis=-1)
    return _gelu64(feats @ W1 + b1) @ W2 + b2


def _tone_grid():
    freqs = np.logspace(math.log10(F_MIN), math.log10(F_MAX), RFF).astype(np.float64)
    uni = np.arange(KU) / L_PER
    cut = uni[-1]
    cand = sorted(set(
        round(f, 6)
        for f in np.concatenate([(freqs[:, None] + freqs[None, :]).ravel(), 2 * freqs])
        if cut + 0.2 < f < 145.0
    ))
    omQ = np.concatenate([uni, np.asarray(cand[: NQMAX - KU], dtype=np.float64)])
    omQ.sort()
    omP = omQ[1:min(len(omQ), 129)]
    return freqs, omQ, omP


def _fit_coefs(phase, W1, b1, W2, b2):
    freqs, omQ, omP = _tone_grid()
    NG = 32768
    Dg = (np.arange(NG) + 0.5) / NG
    G = _g_of_D(Dg, phase, W1, b1, W2, b2, freqs)
    Phi = np.concatenate(
        [np.cos(Dg[:, None] * TWO_PI * omQ[None, :]),
         np.sin(Dg[:, None] * TWO_PI * omP[None, :])], axis=1)
    lam = FIT_LAM * NG
    w = np.ones(NG)
    best = None
    for _ in range(FIT_ITERS):
        Pw = Phi * w[:, None]
        A = Pw.T @ Phi
        A[np.diag_indices_from(A)] += lam
        coef = np.linalg.solve(A, Pw.T @ G)
        res = np.abs(Phi @ coef - G).max(axis=1)
        mx = res.max()
        if best is None or mx < best[0]:
            best = (mx, coef.copy())
        w = w * (0.05 + res / mx)
        w = np.maximum(w / w.mean(), 1e-6)
    mx, coef = best
    return omQ, omP, coef[: len(omQ)], coef[len(omQ):], mx


# ---------------------------------------------------------------- device
def _build_module():
    import concourse.tile as tile
    from concourse import bacc, mybir
    from contextlib import ExitStack

    f32 = mybir.dt.float32
    bf16 = mybir.dt.bfloat16
    Alu = mybir.AluOpType
    Act = mybir.ActivationFunctionType

    nc = bacc.Bacc("TRN2", target_bir_lowering=False, debug=False)

    # packU = [U | V] bf16: U[p, c*512+s], V = pair-swapped U
    packU_d = nc.dram_tensor("packU", [TILE, 4 * T], bf16, kind="ExternalInput")
    # packC f32: cols 0:32 = aq (col 2h+c), cols 32:64 = bq
    packC_d = nc.dram_tensor("packC", [TILE, 4 * NH], f32, kind="ExternalInput")
    # compact block-lower-triangle outputs, head-major per row-tile i
    out_ds = [nc.dram_tensor(f"out{i}", [TILE, NH * (i + 1) * TILE], bf16,
                             kind="ExternalOutput") for i in range(NT)]

    with tile.TileContext(nc) as tc:
        with ExitStack() as ctx:
            const = ctx.enter_context(tc.tile_pool(name="const", bufs=1))
            rhspool = ctx.enter_context(tc.tile_pool(name="rhs", bufs=1))
            tmppool = ctx.enter_context(tc.tile_pool(name="tmp", bufs=6))
            stpool = ctx.enter_context(tc.tile_pool(name="stage", bufs=1))
            # PSUM: pb = [128,1024] (2 banks) x2 bufs for i3/i2 head pairs;
            # pv = [128,1536] (3 banks) x1 buf for i1 pairs + i0;
            # pw = [128,512] x1 buf for warmup.  4+3+1 = 8 banks.
            pb = ctx.enter_context(tc.tile_pool(name="pb", bufs=2, space="PSUM"))
            pv = ctx.enter_context(tc.tile_pool(name="pv", bufs=1, space="PSUM"))
            pw = ctx.enter_context(tc.tile_pool(name="pw", bufs=1, space="PSUM"))

            packU = const.tile([TILE, 4 * T], bf16, tag="packU")
            nc.sync.dma_start(packU[:], packU_d.ap())
            packC = const.tile([TILE, 4 * NH], f32, tag="packC")
            nc.scalar.dma_start(packC[:], packC_d.ap())

            U = [packU[:, c * T:(c + 1) * T] for c in range(2)]
            V = [packU[:, (2 + c) * T:(3 + c) * T] for c in range(2)]
            aq = packC[:, 0:2 * NH]
            bq = packC[:, 2 * NH:4 * NH]

            # PE warm-up: ~3us of dummy matmuls to finish the p-state ramp
            # while the input DMA + first builds land.
            warm_sb = const.tile([TILE, T], bf16, tag="warm")
            nc.vector.memset(warm_sb[:], 0)
            for _ in range(NWARM):
                warm_ps = pw.tile([TILE, T], f32, tag="pw")
                nc.tensor.matmul(warm_ps[:], warm_sb[:, 0:TILE], warm_sb[:],
                                 start=True, stop=True)

            # rcomb: [128, NH*1024], per head h chunk c at h*1024 + c*512
            rcomb = rhspool.tile([TILE, NH * 2 * T], bf16, tag="rcomb",
                                 name="rcomb")

            # 3-op build, all ops 2x-eligible: t1 = U(.)aq (ts), t2 = V(.)bq
            # (ts or Act scale-identity), rcomb = t1 + t2 (tt add).
            # ~11 of the 64 ts ops ride the Scalar engine to balance load.
            def build(h):
                for c in range(2):
                    col = 2 * h + c
                    t1 = tmppool.tile([TILE, T], bf16, tag="tmp")
                    t2 = tmppool.tile([TILE, T], bf16, tag="tmp")
                    nc.vector.tensor_scalar(
                        t1[:], U[c], aq[:, col:col + 1], None, Alu.mult)
                    if (2 * h + c) % 6 == 1:
                        nc.scalar.activation(t2[:], V[c], Act.Identity,
                                             scale=bq[:, col:col + 1])
                    else:
                        nc.vector.tensor_scalar(
                            t2[:], V[c], bq[:, col:col + 1], None, Alu.mult)
                    nc.vector.tensor_tensor(
                        rcomb[:, h * 2 * T + c * T: h * 2 * T + (c + 1) * T],
                        t1[:], t2[:], Alu.add)

            for h in range(4):
                build(h)

            stages = [stpool.tile([TILE, NH * (i + 1) * TILE], bf16,
                                  tag=f"stage{i}", name=f"stage{i}")
                      for i in range(NT)]
            rc_v = rcomb[:].rearrange("p (h x) -> p h x", h=NH)

            def rslice(h, c, n):
                return rcomb[:, h * 2 * T + c * T: h * 2 * T + c * T + n]

            W2_ = 3 * TILE
            W1_ = 2 * TILE
            for g in range(4):
                h0 = 4 * g
                # i = 3: head pairs, 2 banks per psum tile, N=512 each
                for hp in (h0, h0 + 2):
                    ps = pb.tile([TILE, 2 * T], f32, tag="pb")
                    for j in range(2):
                        for c in range(2):
                            nc.tensor.matmul(
                                ps[:, j * T:(j + 1) * T],
                                packU[:, c * T + 3 * TILE: c * T + 4 * TILE],
                                rslice(hp + j, c, T),
                                start=(c == 0), stop=(c == 1),
                                skip_group_check=True)
                    nc.scalar.activation(
                        stages[3][:, hp * T:(hp + 2) * T], ps[:], Act.Identity)
                # builds for the next head group while PE chews on i1/i0/i2
                if g < 3:
                    for h in range(h0 + 4, h0 + 8):
                        build(h)
                # i = 1 (two heads per bank) + i = 0 (four heads in one bank)
                psv = pv.tile([TILE, 3 * T], f32, tag="pv")
                for k, hp in enumerate((h0, h0 + 2)):
                    for c in range(2):
                        nc.tensor.matmul(
                            psv[:, k * T:(k + 1) * T],
                            packU[:, c * T + TILE: c * T + 2 * TILE],
                            rc_v[:, hp:hp + 2, c * T: c * T + W1_],
                            start=(c == 0), stop=(c == 1),
                            skip_group_check=True)
                for c in range(2):
                    nc.tensor.matmul(
                        psv[:, 2 * T:3 * T],
                        packU[:, c * T: c * T + TILE],
                        rc_v[:, h0:h0 + 4, c * T: c * T + TILE],
                        start=(c == 0), stop=(c == 1),
                        skip_group_check=True)
                nc.scalar.activation(
                    stages[1][:, h0 * W1_:(h0 + 4) * W1_], psv[:, 0:2 * T],
                    Act.Identity)
                nc.scalar.activation(
                    stages[0][:, h0 * TILE:(h0 + 4) * TILE], psv[:, 2 * T:3 * T],
                    Act.Identity)
                nc.sync.dma_start(
                    out_ds[3].ap()[:, h0 * T:(h0 + 4) * T],
                    stages[3][:, h0 * T:(h0 + 4) * T])
                # i = 2: head pairs, N=384 into each bank of a 2-bank tile
                for hp in (h0, h0 + 2):
                    ps = pb.tile([TILE, 2 * T], f32, tag="pb")
                    for j in range(2):
                        for c in range(2):
                            nc.tensor.matmul(
                                ps[:, j * T:j * T + W2_],
                                packU[:, c * T + 2 * TILE: c * T + 3 * TILE],
                                rslice(hp + j, c, W2_),
                                start=(c == 0), stop=(c == 1),
                                skip_group_check=True)
                    nc.scalar.activation(
                        stages[2][:, hp * W2_:(hp + 2) * W2_]
                        .rearrange("p (b s) -> p b s", b=2),
                        ps[:].rearrange("p (b s) -> p b s", b=2)[:, :, 0:W2_],
                        Act.Identity)
                nc.sync.dma_start(
                    out_ds[1].ap()[:, h0 * W1_:(h0 + 4) * W1_],
                    stages[1][:, h0 * W1_:(h0 + 4) * W1_])
                nc.sync.dma_start(
                    out_ds[0].ap()[:, h0 * TILE:(h0 + 4) * TILE],
                    stages[0][:, h0 * TILE:(h0 + 4) * TILE])
                nc.sync.dma_start(
                    out_ds[2].ap()[:, h0 * W2_:(h0 + 4) * W2_],
                    stages[2][:, h0 * W2_:(h0 + 4) * W2_])

    nc.compile()
    return nc


# ---------------------------------------------------------------- host glue
def _to_bf16(x):
    import ml_dtypes
    return np.ascontiguousarray(x, np.float32).astype(ml_dtypes.bfloat16)


def _host_tables(c_sorted, omQ):
    """U: [128, 1024] interleaved cos/sin, chunk-major; V: pair-swapped."""
    nQ = len(omQ)
    ang = np.multiply.outer(omQ, c_sorted.astype(np.float64)) * TWO_PI  # [nQ, T]
    cosr = np.cos(ang).astype(np.float32)
    sinr = np.sin(ang).astype(np.float32)
    U = np.zeros((TILE, 2 * T), np.float32)
    for c in range(2):
        for kk in range(64):
            k = 64 * c + kk
            if k >= nQ:
                break
            U[2 * kk, c * T:(c + 1) * T] = cosr[k]
            U[2 * kk + 1, c * T:(c + 1) * T] = sinr[k]
    Vt = np.zeros_like(U)
    Vt[0::2] = U[1::2]
    Vt[1::2] = U[0::2]
    return U, Vt


def _coef_cols(a, b, nQ):
    """packC [128, 64] f32: cols 0:32 aq (col 2h+c), 32:64 bq (-b even, +b odd)."""
    a_pad = np.zeros((TILE, NH), np.float64)
    a_pad[:nQ] = a
    b_pad = np.zeros((TILE, NH), np.float64)
    b_pad[1:1 + b.shape[0]] = b
    pc = np.zeros((TILE, 4 * NH), np.float32)
    for h in range(NH):
        for c in range(2):
            col = 2 * h + c
            for kk in range(64):
                k = 64 * c + kk
                pc[2 * kk, col] = a_pad[k, h]
                pc[2 * kk + 1, col] = a_pad[k, h]
                pc[2 * kk, 2 * NH + col] = -b_pad[k, h]
                pc[2 * kk + 1, 2 * NH + col] = b_pad[k, h]
    return pc


def kernel(centers01, mask, bias_phase, W1, b1, W2, b2):
    global _MODULE, _LAST_RESULTS
    from concourse.bass_utils import run_bass_kernel_spmd

    centers01 = np.asarray(centers01, np.float32)
    bias_phase = np.asarray(bias_phase, np.float64)
    W1 = np.asarray(W1, np.float64)
    b1 = np.asarray(b1, np.float64)
    W2 = np.asarray(W2, np.float64)
    b2 = np.asarray(b2, np.float64)

    ck = hash((bias_phase.tobytes(), W1.tobytes(), b1.tobytes(),
               W2.tobytes(), b2.tobytes()))
    if ck not in _FIT_CACHE:
        _FIT_CACHE[ck] = _fit_coefs(bias_phase, W1, b1, W2, b2)
    omQ, omP, a, b, _gridmax = _FIT_CACHE[ck]
    nQ = len(omQ)

    packC = _coef_cols(a, b, nQ)

    if _MODULE is None:
        _MODULE = _build_module()
    nc = _MODULE

    in_maps = []
    idxs = []
    for bi in range(N_CORES):
        c = centers01[bi]
        idx = np.argsort(c, kind="stable")
        idxs.append(idx)
        U, Vt = _host_tables(c[idx], omQ)
        in_maps.append({
            "packU": _to_bf16(np.concatenate([U, Vt], axis=1)),
            "packC": packC,
        })

    res = run_bass_kernel_spmd(nc, in_maps, list(range(N_CORES)))
    _LAST_RESULTS = res

    out = np.empty((B, NH, T, T), np.float32)
    M = np.empty((NH, T, T), np.float32)
    iu = np.triu_indices(T, 1)
    for bi in range(N_CORES):
        for i in range(NT):
            Wi = (i + 1) * TILE
            raw = np.asarray(res.results[bi][f"out{i}"])
            if raw.dtype != np.uint16:
                raw = raw.view(np.uint16)
            f = (raw.astype(np.uint32) << 16).view(np.float32)
            M[:, i * TILE:(i + 1) * TILE, 0:Wi] = \
                f.reshape(TILE, NH, Wi).transpose(1, 0, 2)
        M[:, iu[0], iu[1]] = M[:, iu[1], iu[0]]
        inv = np.empty(T, np.int64)
        inv[idxs[bi]] = np.arange(T)
        out[bi] = M[:, inv][:, :, inv]
    m = np.asarray(mask, bool)
    if not m.all():
        out *= (m[:, None, :, None] & m[:, None, None, :]).astype(np.float32)
    return out
